# revision 4
# baseline (speedup 1.0000x reference)
"""CRF Viterbi decode kernel for 8 Trainium2 NeuronCores.

Layout (per core, 32 sequences):
  partition p = 4*b_loc + q  (b_loc in [0,32), q in [0,4))
  partition p owns cur tags c in [16q, 16q+16)  (CL=16 per partition)

Forward step s (t = s+1), per partition:
  SC[c',pv] = trans[pv, 16q+c'] + alpha[b, pv]        (TT add, [128,16,64])
  M[c']     = max_pv SC                               (tensor_reduce X)
  D         = M - SC                                  (TT sub)
  E         = D*2^36 + iota[pv]                       (scalar_tensor_tensor)
  bp[c']    = min_pv E                                (tensor_reduce X)  == first argmax, exact
  alpha'    = max(alpha + PEN[s], M + XP[s])          (TT add + STT)
where XP = x[:,1:] with -1e30 added beyond seq_len, PEN = -1e30 on valid
steps / 0 beyond seq_len (freezes alpha exactly).

alpha quarters -> full-alpha-per-partition via 4 one-hot PE matmuls into PSUM.
Backpointers (uint8) stream to DRAM; the O(B*T) backtrace runs on host.
"""

import sys
from contextlib import ExitStack

import numpy as np

sys.path.insert(0, "/opt/trn_rl_repo")

import concourse.bacc as bacc  # noqa: E402
import concourse.tile as tile  # noqa: E402
from concourse import bass_utils, mybir  # noqa: E402
from concourse.bass import ds  # noqa: E402

B, T, N = 256, 2048, 64
NCORES = 8
BPC = B // NCORES  # 32 sequences per core
P = 128
QN = 4             # cur-quarters per sequence
CL = N // QN       # 16 cur tags per partition
NEG = -1.0e30
BIG = float(2.0 ** 36)

F32 = mybir.dt.float32
U8 = mybir.dt.uint8
ALU = mybir.AluOpType
AX = mybir.AxisListType
ET = mybir.EngineType


def build_module(ts=T, u=64):
    """Build + compile the SPMD module. ts = padded step count (multiple of u)."""
    assert ts % u == 0
    nc = bacc.Bacc(
        "TRN2", target_bir_lowering=False, debug=False, num_devices=NCORES
    )
    xpl = nc.dram_tensor("xpl", [P, ts, CL], F32, kind="ExternalInput").ap()
    penp = nc.dram_tensor("penp", [P, ts], F32, kind="ExternalInput").ap()
    a0 = nc.dram_tensor("alpha0", [P, CL], F32, kind="ExternalInput").ap()
    tflat = nc.dram_tensor("tflat", [P, CL, N], F32, kind="ExternalInput").ap()
    iot = nc.dram_tensor("iota", [P, N], F32, kind="ExternalInput").ap()
    rmat = nc.dram_tensor("rmat", [QN, P, P], F32, kind="ExternalInput").ap()
    bpl = nc.dram_tensor("bpl", [P, ts, CL], U8, kind="ExternalOutput").ap()
    alphaf = nc.dram_tensor("alphaf", [P, CL], F32, kind="ExternalOutput").ap()

    with tile.TileContext(nc) as tc, ExitStack() as ctx:
        singles = ctx.enter_context(tc.tile_pool(name="singles", bufs=1))
        xpool = ctx.enter_context(tc.tile_pool(name="xpool", bufs=3))
        bpool = ctx.enter_context(tc.tile_pool(name="bpool", bufs=3))
        work = ctx.enter_context(tc.tile_pool(name="work", bufs=3))
        mpool = ctx.enter_context(tc.tile_pool(name="mpool", bufs=4))
        psum = ctx.enter_context(tc.tile_pool(name="psum", bufs=2, space="PSUM"))

        tf = singles.tile([P, CL, N], F32, tag="tf")
        nc.sync.dma_start(out=tf, in_=tflat)
        io_t = singles.tile([P, N], F32, tag="iota")
        nc.sync.dma_start(out=io_t, in_=iot)
        rq = []
        for q in range(QN):
            r = singles.tile([P, P], F32, tag=f"rq{q}")
            nc.sync.dma_start(out=r, in_=rmat[q])
            rq.append(r)
        aq = singles.tile([P, CL], F32, tag="aq")
        nc.sync.dma_start(out=aq, in_=a0)

        with tc.For_i(0, ts, u, hint_engines=(ET.DVE, ET.PE)) as base:
            xt = xpool.tile([P, u, CL], F32, tag="xt")
            nc.sync.dma_start(out=xt, in_=xpl[:, ds(base, u), :])
            pt = xpool.tile([P, u], F32, tag="pt")
            nc.sync.dma_start(out=pt, in_=penp[:, ds(base, u)])
            bpt = bpool.tile([P, u, CL], U8, tag="bpt")
            for s in range(u):
                sfull = psum.tile([P, N], F32, tag="sfull")
                for q in range(QN):
                    nc.tensor.matmul(
                        out=sfull[:, q * CL:(q + 1) * CL],
                        lhsT=rq[q],
                        rhs=aq,
                        start=True,
                        stop=True,
                    )
                sc = work.tile([P, CL, N], F32, tag="sc")
                nc.vector.tensor_tensor(
                    out=sc,
                    in0=tf,
                    in1=sfull.unsqueeze(1).broadcast_to([P, CL, N]),
                    op=ALU.add,
                )
                m = mpool.tile([P, CL], F32, tag="m")
                nc.vector.tensor_reduce(out=m, in_=sc, axis=AX.X, op=ALU.max)
                d = work.tile([P, CL, N], F32, tag="d")
                nc.vector.tensor_tensor(
                    out=d,
                    in0=m.unsqueeze(2).broadcast_to([P, CL, N]),
                    in1=sc,
                    op=ALU.subtract,
                )
                e = work.tile([P, CL, N], F32, tag="e")
                nc.vector.scalar_tensor_tensor(
                    out=e,
                    in0=d,
                    scalar=BIG,
                    in1=io_t.unsqueeze(1).broadcast_to([P, CL, N]),
                    op0=ALU.mult,
                    op1=ALU.add,
                )
                bpf = mpool.tile([P, CL], F32, tag="bpf")
                nc.vector.tensor_reduce(out=bpf, in_=e, axis=AX.X, op=ALU.min)
                nc.vector.tensor_copy(out=bpt[:, s, :], in_=bpf)
                nraw = mpool.tile([P, CL], F32, tag="nraw")
                nc.vector.tensor_tensor(
                    out=nraw, in0=m, in1=xt[:, s, :], op=ALU.add
                )
                nc.vector.scalar_tensor_tensor(
                    out=aq,
                    in0=aq,
                    scalar=pt[:, s:s + 1],
                    in1=nraw,
                    op0=ALU.add,
                    op1=ALU.max,
                )
            nc.sync.dma_start(out=bpl[:, ds(base, u), :], in_=bpt)
        nc.sync.dma_start(out=alphaf, in_=aq)

    nc.compile()
    return nc


def prep_inputs(x, seq_len, trans, ts=T):
    """Host-side preprocessing -> per-core in_maps."""
    x = np.asarray(x, dtype=np.float32)
    seq_len = np.asarray(seq_len, dtype=np.int32)
    trans = np.asarray(trans, dtype=np.float32)
    b, t, n = x.shape

    # shared tables
    tflat = np.ascontiguousarray(
        np.broadcast_to(
            trans.T.reshape(QN, CL, N)[None, :, :, :], (BPC, QN, CL, N)
        ).reshape(P, CL, N)
    ).astype(np.float32)  # tflat[4b+q, c', pv] = trans[pv, 16q+c']
    iota = np.broadcast_to(
        np.arange(N, dtype=np.float32)[None, :], (P, N)
    ).copy()
    rmat = np.zeros((QN, P, P), dtype=np.float32)
    mm = np.arange(P)
    for q in range(QN):
        rmat[q, 4 * (mm // 4) + q, mm] = 1.0

    # validity: step s updates alpha with x[:, s+1]; valid iff (s+1) < L
    steps = np.arange(1, ts + 1)[None, :]  # t = s+1
    validf = steps < seq_len[:, None]  # [B, ts]; t >= T rows auto-invalid
    xpen = np.where(validf, np.float32(0.0), np.float32(NEG))  # [B, ts]
    pen_alpha = np.where(validf, np.float32(NEG), np.float32(0.0))

    in_maps = []
    for k in range(NCORES):
        sl = slice(k * BPC, (k + 1) * BPC)
        xc = x[sl]  # [32, T, N]
        # XP[s] = x[:, s+1] + xpen; pad steps beyond T-1 with NEG
        xp = np.full((BPC, ts, N), NEG, dtype=np.float32)
        nreal = min(ts, t - 1)
        xp[:, :nreal] = xc[:, 1:1 + nreal]
        xp += xpen[sl][:, :, None]
        xp = np.minimum(xp, np.float32(3.4e38))  # avoid -inf overflow; keeps NEG
        # -> [P, ts, CL]: partition 4b+q gets cur-quarter q
        xpl = np.ascontiguousarray(
            xp.reshape(BPC, ts, QN, CL).transpose(0, 2, 1, 3).reshape(P, ts, CL)
        )
        penp = np.ascontiguousarray(
            np.broadcast_to(pen_alpha[sl][:, None, :], (BPC, QN, ts)).reshape(
                P, ts
            )
        )
        a0 = np.ascontiguousarray(
            xc[:, 0].reshape(BPC, QN, CL).reshape(P, CL)
        )
        in_maps.append(
            {
                "xpl": xpl,
                "penp": penp,
                "alpha0": a0,
                "tflat": tflat,
                "iota": iota,
                "rmat": rmat,
            }
        )
    return in_maps


def postprocess(results, seq_len, t_real=T):
    """results: per-core dicts with 'bpl' [P,ts,CL] u8 and 'alphaf' [P,CL]."""
    seq_len = np.asarray(seq_len, dtype=np.int32)
    bps = []
    alphas = []
    for r in results:
        bpl = r["bpl"]
        ts = bpl.shape[1]
        bp = (
            bpl.reshape(BPC, QN, ts, CL)
            .transpose(0, 2, 1, 3)
            .reshape(BPC, ts, N)
            .astype(np.int32)
        )
        bps.append(bp)
        alphas.append(r["alphaf"].reshape(BPC, QN * CL))
    bp_all = np.concatenate(bps, axis=0)  # [B, ts, N], step s == t-1
    alpha_f = np.concatenate(alphas, axis=0)  # [B, N]

    scores = alpha_f.max(axis=-1)
    last = alpha_f.argmax(axis=-1).astype(np.int32)

    b = alpha_f.shape[0]
    tags = np.empty((b, t_real), dtype=np.int32)
    tags[:, t_real - 1] = last
    tag = last.copy()
    ar = np.arange(b)
    for t in range(t_real - 2, -1, -1):
        nxt = bp_all[:, t, :][ar, tag]  # backpointer of step t+1
        msk = (t + 1) < seq_len
        tag = np.where(msk, nxt, tag).astype(np.int32)
        tags[:, t] = tag
    return tags, scores


_module_cache = {}


def _get_module(ts=T, u=64):
    key = (ts, u)
    if key not in _module_cache:
        _module_cache[key] = build_module(ts, u)
    return _module_cache[key]


def kernel(x, seq_len, trans):
    nc = _get_module()
    in_maps = prep_inputs(x, seq_len, trans)
    res = bass_utils.run_bass_kernel_spmd(
        nc, in_maps, core_ids=list(range(NCORES))
    )
    return postprocess(res.results, seq_len)


# revision 8
# speedup vs baseline: 1.0793x; 1.0793x over previous
"""CRF Viterbi decode kernel for 8 Trainium2 NeuronCores.

Layout (per core, 32 sequences):
  partition p = 4*b_loc + q  (b_loc in [0,32), q in [0,4))
  partition p owns cur tags c in [16q, 16q+16)  (CL=16 per partition)

Forward step s (t = s+1), per partition:
  SC[c',pv] = trans[pv, 16q+c'] + alpha[b, pv]        (TT add, [128,16,64])
  M[c']     = max_pv SC                               (tensor_reduce X)
  D         = M - SC                                  (TT sub)
  E         = D*2^36 + iota[pv]                       (scalar_tensor_tensor)
  bp[c']    = min_pv E                                (tensor_reduce X)  == first argmax, exact
  alpha'    = max(alpha + PEN[s], M + XP[s])          (TT add + STT)
where XP = x[:,1:] with -1e30 added beyond seq_len, PEN = -1e30 on valid
steps / 0 beyond seq_len (freezes alpha exactly).

alpha quarters -> full-alpha-per-partition via 4 one-hot PE matmuls into PSUM.
Backpointers (uint8) stream to DRAM; the O(B*T) backtrace runs on host.
"""

import sys
from contextlib import ExitStack

import numpy as np

sys.path.insert(0, "/opt/trn_rl_repo")

import concourse.bacc as bacc  # noqa: E402
import concourse.tile as tile  # noqa: E402
from concourse import bass_utils, mybir  # noqa: E402
from concourse.bass import ds  # noqa: E402

B, T, N = 256, 2048, 64
NCORES = 8
BPC = B // NCORES  # 32 sequences per core
P = 128
QN = 4             # cur-quarters per sequence
CL = N // QN       # 16 cur tags per partition
NEG = -1.0e30
BIG = float(2.0 ** 36)

F32 = mybir.dt.float32
U8 = mybir.dt.uint8
ALU = mybir.AluOpType
AX = mybir.AxisListType
ET = mybir.EngineType


def build_module(ts=T, u=64, variant="full"):
    """Build + compile the SPMD module. ts = padded step count (multiple of u).
    variant: timing-bisect builds ("full", "noargmax", "nopsum", "nomm")."""
    assert ts % u == 0
    nc = bacc.Bacc(
        "TRN2", target_bir_lowering=False, debug=False, num_devices=NCORES
    )
    xpl = nc.dram_tensor("xpl", [P, ts, CL], F32, kind="ExternalInput").ap()
    penp = nc.dram_tensor("penp", [P, ts], F32, kind="ExternalInput").ap()
    a0 = nc.dram_tensor("alpha0", [P, CL], F32, kind="ExternalInput").ap()
    tflat = nc.dram_tensor("tflat", [P, CL, N], F32, kind="ExternalInput").ap()
    iot = nc.dram_tensor("iota", [P, N], F32, kind="ExternalInput").ap()
    rmat = nc.dram_tensor("rmat", [QN, P, P], F32, kind="ExternalInput").ap()
    bpl = nc.dram_tensor("bpl", [P, ts, CL], U8, kind="ExternalOutput").ap()
    alphaf = nc.dram_tensor("alphaf", [P, CL], F32, kind="ExternalOutput").ap()

    with tile.TileContext(nc) as tc, ExitStack() as ctx:
        singles = ctx.enter_context(tc.tile_pool(name="singles", bufs=1))
        xpool = ctx.enter_context(tc.tile_pool(name="xpool", bufs=3))
        bpool = ctx.enter_context(tc.tile_pool(name="bpool", bufs=3))
        work = ctx.enter_context(tc.tile_pool(name="work", bufs=3))
        mpool = ctx.enter_context(tc.tile_pool(name="mpool", bufs=4))
        psum = ctx.enter_context(tc.tile_pool(name="psum", bufs=2, space="PSUM"))

        tf = singles.tile([P, CL, N], F32, tag="tf")
        nc.sync.dma_start(out=tf, in_=tflat)
        io_t = singles.tile([P, N], F32, tag="iota")
        nc.sync.dma_start(out=io_t, in_=iot)
        rq = []
        for q in range(QN):
            r = singles.tile([P, P], F32, tag=f"rq{q}")
            nc.sync.dma_start(out=r, in_=rmat[q])
            rq.append(r)
        aq = singles.tile([P, CL], F32, tag="aq")
        nc.sync.dma_start(out=aq, in_=a0)

        with tc.For_i(0, ts, u, hint_engines=(ET.DVE, ET.PE)) as base:
            xt = xpool.tile([P, u, CL], F32, tag="xt")
            nc.sync.dma_start(out=xt, in_=xpl[:, ds(base, u), :])
            pt = xpool.tile([P, u], F32, tag="pt")
            nc.sync.dma_start(out=pt, in_=penp[:, ds(base, u)])
            bpt = bpool.tile([P, u, CL], U8, tag="bpt") if variant == "full" else None
            for s in range(u):
                if variant != "nomm":
                    sfull = psum.tile([P, N], F32, tag="sfull")
                    for q in range(QN):
                        nc.tensor.matmul(
                            out=sfull[:, q * CL:(q + 1) * CL],
                            lhsT=rq[q],
                            rhs=aq,
                            start=True,
                            stop=True,
                        )
                sc = work.tile([P, CL, N], F32, tag="sc")
                if variant in ("nopsum", "nomm"):
                    sc_in1 = tf
                else:
                    sc_in1 = sfull.unsqueeze(1).broadcast_to([P, CL, N])
                nc.vector.tensor_tensor(out=sc, in0=tf, in1=sc_in1, op=ALU.add)
                m = mpool.tile([P, CL], F32, tag="m")
                nc.vector.tensor_reduce(out=m, in_=sc, axis=AX.X, op=ALU.max)
                if variant == "full":
                    d = work.tile([P, CL, N], F32, tag="d")
                    nc.vector.tensor_tensor(
                        out=d,
                        in0=m.unsqueeze(2).broadcast_to([P, CL, N]),
                        in1=sc,
                        op=ALU.subtract,
                    )
                    e = work.tile([P, CL, N], F32, tag="e")
                    nc.vector.scalar_tensor_tensor(
                        out=e,
                        in0=d,
                        scalar=BIG,
                        in1=io_t.unsqueeze(1).broadcast_to([P, CL, N]),
                        op0=ALU.mult,
                        op1=ALU.add,
                    )
                    bpf = mpool.tile([P, CL], F32, tag="bpf")
                    nc.vector.tensor_reduce(
                        out=bpf, in_=e, axis=AX.X, op=ALU.min
                    )
                    nc.vector.tensor_copy(out=bpt[:, s, :], in_=bpf)
                nraw = mpool.tile([P, CL], F32, tag="nraw")
                nc.vector.tensor_tensor(
                    out=nraw, in0=m, in1=xt[:, s, :], op=ALU.add
                )
                nc.vector.scalar_tensor_tensor(
                    out=aq,
                    in0=aq,
                    scalar=pt[:, s:s + 1],
                    in1=nraw,
                    op0=ALU.add,
                    op1=ALU.max,
                )
            if variant == "full":
                nc.sync.dma_start(out=bpl[:, ds(base, u), :], in_=bpt)
        nc.sync.dma_start(out=alphaf, in_=aq)

    nc.compile()
    return nc


def prep_inputs(x, seq_len, trans, ts=T):
    """Host-side preprocessing -> per-core in_maps."""
    x = np.asarray(x, dtype=np.float32)
    seq_len = np.asarray(seq_len, dtype=np.int32)
    trans = np.asarray(trans, dtype=np.float32)
    b, t, n = x.shape

    # shared tables
    tflat = np.ascontiguousarray(
        np.broadcast_to(
            trans.T.reshape(QN, CL, N)[None, :, :, :], (BPC, QN, CL, N)
        ).reshape(P, CL, N)
    ).astype(np.float32)  # tflat[4b+q, c', pv] = trans[pv, 16q+c']
    iota = np.broadcast_to(
        np.arange(N, dtype=np.float32)[None, :], (P, N)
    ).copy()
    rmat = np.zeros((QN, P, P), dtype=np.float32)
    mm = np.arange(P)
    for q in range(QN):
        rmat[q, 4 * (mm // 4) + q, mm] = 1.0

    # validity: step s updates alpha with x[:, s+1]; valid iff (s+1) < L
    steps = np.arange(1, ts + 1)[None, :]  # t = s+1
    validf = steps < seq_len[:, None]  # [B, ts]; t >= T rows auto-invalid
    xpen = np.where(validf, np.float32(0.0), np.float32(NEG))  # [B, ts]
    pen_alpha = np.where(validf, np.float32(NEG), np.float32(0.0))

    in_maps = []
    for k in range(NCORES):
        sl = slice(k * BPC, (k + 1) * BPC)
        xc = x[sl]  # [32, T, N]
        # XP[s] = x[:, s+1] + xpen; pad steps beyond T-1 with NEG
        xp = np.full((BPC, ts, N), NEG, dtype=np.float32)
        nreal = min(ts, t - 1)
        xp[:, :nreal] = xc[:, 1:1 + nreal]
        xp += xpen[sl][:, :, None]
        xp = np.minimum(xp, np.float32(3.4e38))  # avoid -inf overflow; keeps NEG
        # -> [P, ts, CL]: partition 4b+q gets cur-quarter q
        xpl = np.ascontiguousarray(
            xp.reshape(BPC, ts, QN, CL).transpose(0, 2, 1, 3).reshape(P, ts, CL)
        )
        penp = np.ascontiguousarray(
            np.broadcast_to(pen_alpha[sl][:, None, :], (BPC, QN, ts)).reshape(
                P, ts
            )
        )
        a0 = np.ascontiguousarray(
            xc[:, 0].reshape(BPC, QN, CL).reshape(P, CL)
        )
        in_maps.append(
            {
                "xpl": xpl,
                "penp": penp,
                "alpha0": a0,
                "tflat": tflat,
                "iota": iota,
                "rmat": rmat,
            }
        )
    return in_maps


def postprocess(results, seq_len, t_real=T):
    """results: per-core dicts with 'bpl' [P,ts,CL] u8 and 'alphaf' [P,CL]."""
    seq_len = np.asarray(seq_len, dtype=np.int32)
    bps = []
    alphas = []
    for r in results:
        bpl = r["bpl"]
        ts = bpl.shape[1]
        bp = (
            bpl.reshape(BPC, QN, ts, CL)
            .transpose(0, 2, 1, 3)
            .reshape(BPC, ts, N)
            .astype(np.int32)
        )
        bps.append(bp)
        alphas.append(r["alphaf"].reshape(BPC, QN * CL))
    bp_all = np.concatenate(bps, axis=0)  # [B, ts, N], step s == t-1
    alpha_f = np.concatenate(alphas, axis=0)  # [B, N]

    scores = alpha_f.max(axis=-1)
    last = alpha_f.argmax(axis=-1).astype(np.int32)

    b = alpha_f.shape[0]
    tags = np.empty((b, t_real), dtype=np.int32)
    tags[:, t_real - 1] = last
    tag = last.copy()
    ar = np.arange(b)
    for t in range(t_real - 2, -1, -1):
        nxt = bp_all[:, t, :][ar, tag]  # backpointer of step t+1
        msk = (t + 1) < seq_len
        tag = np.where(msk, nxt, tag).astype(np.int32)
        tags[:, t] = tag
    return tags, scores


_module_cache = {}


def _get_module(ts=T, u=64):
    key = (ts, u)
    if key not in _module_cache:
        _module_cache[key] = build_module(ts, u)
    return _module_cache[key]


def kernel(x, seq_len, trans):
    nc = _get_module()
    in_maps = prep_inputs(x, seq_len, trans)
    res = bass_utils.run_bass_kernel_spmd(
        nc, in_maps, core_ids=list(range(NCORES))
    )
    return postprocess(res.results, seq_len)
